# revision 31
# baseline (speedup 1.0000x reference)
"""Trainium2 Bass kernel for nn_ExpertParallelFrontBlock (MoE top-2 routing + front FFN).

Expert-parallel: 1 expert per NeuronCore (8 cores). Only the discrete top-2
decisions matter for the output (dispatch uses the 0/1 slot mask, not softmax
probs), so no softmax is computed.

Per core:
  1. Gate logits for all 4096 tokens, exact 3-pass bf16 hi/lo (top-1/2 margins
     are ~1e-5 so the router needs fp32-grade logits). The hi and lo gate
     columns ride one stationary (PE cost is N cycles regardless of M), so it
     is 2 matmuls per d-tile, not 3. x arrives host-pre-split/pre-transposed.
     Per-chunk top-2 mask/chosen extraction is fused into the stream loop.
  2. Routing: rank-in-expert via UT128 cumsum matmul + cross-tile carry
     (bf16 operands, exact integer math); capacity drop falls out of the
     slot->token inverse map, built with fp16 one-hot matmuls whose token id
     is carried as exact (p, t, valid) columns.
  3. Dispatch: indirect-DMA row gather from bf16 x (empty slots read an
     appended zero row), transposed on the PE per c-tile.
  4. FFN [640,1024]@[1024,4096]+bias in single-pass bf16 (output tolerance is
     2e-2; 1-pass is ~4e-3) with the bias added on the vector engine during
     PSUM eviction; bf16 output is upcast on host.

Host pre-formats all operands (bf16 hi/lo splits, transposes, per-chunk
contiguous layouts, replicated bias); weights/bias/x staged per core.

self-contained: hardcodes all shapes from the problem spec.
"""
import numpy as np
import ml_dtypes

import concourse.bass as bass
import concourse.mybir as mybir
import concourse.tile as tile
from concourse import bacc
from concourse.bass_utils import run_bass_kernel_spmd

F32 = mybir.dt.float32
BF16 = mybir.dt.bfloat16
I32 = mybir.dt.int32
F16 = mybir.dt.float16

S, D, E, F = 4096, 1024, 8, 4096
C = 640  # capacity: floor(1.25*4096/8) = 640 (even)
P = 128
S_TILES = S // P      # 32
D_TILES = D // P      # 8
C_TILES = C // P      # 5
FCH = 512
F_CHUNKS = F // FCH   # 8
SCH = 512
S_CHUNKS = S // SCH   # 8
SENT = 100000.0

_BUILT = {}


def _build():
    nc = bacc.Bacc("TRN2", target_bir_lowering=False, debug=False, num_devices=E)

    xth = nc.dram_tensor("xth", [P, S_CHUNKS * D_TILES * SCH], BF16, kind="ExternalInput")
    xtl = nc.dram_tensor("xtl", [P, S_CHUNKS * D_TILES * SCH], BF16, kind="ExternalInput")
    xbh = nc.dram_tensor("xbh", [S + P, D], BF16, kind="ExternalInput")
    ghlT_in = nc.dram_tensor("ghlT", [D, 40], BF16, kind="ExternalInput")
    wh = nc.dram_tensor("wh", [D, F], BF16, kind="ExternalInput")
    biasbc_in = nc.dram_tensor("biasbc", [P, F], BF16, kind="ExternalInput")
    selbig = nc.dram_tensor("selbig", [P, S_TILES * E], F32, kind="ExternalInput")
    ident_in = nc.dram_tensor("ident", [P, P], F32, kind="ExternalInput")
    identb_in = nc.dram_tensor("identb", [P, P], BF16, kind="ExternalInput")
    ut128_in = nc.dram_tensor("ut128", [P, P], BF16, kind="ExternalInput")
    mcarry_in = nc.dram_tensor("mcarry", [2 * S_TILES, 2 * S_TILES], F32, kind="ExternalInput")
    ones64_in = nc.dram_tensor("ones64", [2 * S_TILES, P], BF16, kind="ExternalInput")
    onescol_in = nc.dram_tensor("onescol", [P, 1], BF16, kind="ExternalInput")
    iota4_in = nc.dram_tensor("iota4", [P, 4 * C], F16, kind="ExternalInput")
    tokones3_in = nc.dram_tensor("tokones3", [P, 3 * S_TILES], F16, kind="ExternalInput")
    out = nc.dram_tensor("out", [C, F], BF16, kind="ExternalOutput")

    with tile.TileContext(nc) as tc:
        with (
            tc.tile_pool(name="const", bufs=1) as cpool,
            tc.tile_pool(name="persist", bufs=1) as ppool,
        ):
            # ---- constants into SBUF ----
            ghl_sb = cpool.tile([P, D_TILES, 40], BF16, name="ghl_sb")
            nc.sync.dma_start(ghl_sb[:],
                              ghlT_in[:, :].rearrange("(dt p) e -> p dt e", p=P))
            ident_sb = cpool.tile([P, P], F32, name="ident_sb")
            nc.sync.dma_start(ident_sb[:], ident_in[:, :])
            identb_sb = cpool.tile([P, P], BF16, name="identb_sb")
            selbig_sb = cpool.tile([P, S_TILES * E], F32, name="selbig_sb")
            nc.sync.dma_start(selbig_sb[:], selbig[:, :])
            ut_sb = cpool.tile([P, P], BF16, name="ut_sb")
            mcarry_sb = cpool.tile([2 * S_TILES, 2 * S_TILES], F32, name="mcarry_sb")
            ones64_sb = cpool.tile([2 * S_TILES, P], BF16, name="ones64_sb")
            onescol_sb = cpool.tile([P, 1], BF16, name="onescol_sb")
            iota4_sb = cpool.tile([P, 4 * C], F16, name="iota4_sb")
            tokones3_sb = cpool.tile([P, 3 * S_TILES], F16, name="tokones3_sb")

            # persistent across phases
            logits_all = ppool.tile([P, S_TILES * E], F32, name="logits_all")
            chosen12 = ppool.tile([P, 2 * S_TILES], F32, name="chosen12")
            ch_bf = ppool.tile([P, 2 * S_TILES], BF16, name="ch_bf")
            wh_sb = ppool.tile([P, D_TILES, F], BF16, name="wh_sb")
            sl_i = ppool.tile([P, C_TILES], I32, name="sl_i")
            dh_cts = [ppool.tile([P, D_TILES, P], BF16, name=f"dhct{ct}")
                      for ct in range(C_TILES)]
            biasbc_sb = ppool.tile([P, F], BF16, name="biasbc_sb")

            # ---- Phase L: gate logits, 3-pass bf16 from host-transposed x ----
            with (
                tc.tile_pool(name="xt", bufs=3) as xt_pool,
                tc.tile_pool(name="plg", bufs=2, space="PSUM") as plg_pool,
                tc.tile_pool(name="ptr2", bufs=2, space="PSUM") as ptr2_pool,
                tc.tile_pool(name="lgt", bufs=2) as lgt_pool,
                tc.tile_pool(name="rts", bufs=2) as rts_pool,
            ):
                CW = D_TILES * SCH
                HW_ = CW // 2
                for g in range(S_CHUNKS):
                    # every chunk as two separate half-tiles per stream so the
                    # PE starts on dt 0-3 while dt 4-7 is still in flight
                    xh_a = xt_pool.tile([P, 4, SCH], BF16, name=f"xh{g}a", tag="xha")
                    nc.sync.dma_start(
                        xh_a[:], xth[:, g * CW:g * CW + HW_]
                        .rearrange("p (dt s) -> p dt s", s=SCH))
                    xl_a = xt_pool.tile([P, 4, SCH], BF16, name=f"xl{g}a", tag="xla")
                    nc.sync.dma_start(
                        xl_a[:], xtl[:, g * CW:g * CW + HW_]
                        .rearrange("p (dt s) -> p dt s", s=SCH))
                    xh_b = xt_pool.tile([P, 4, SCH], BF16, name=f"xh{g}b", tag="xhb")
                    nc.sync.dma_start(
                        xh_b[:], xth[:, g * CW + HW_:(g + 1) * CW]
                        .rearrange("p (dt s) -> p dt s", s=SCH))
                    xl_b = xt_pool.tile([P, 4, SCH], BF16, name=f"xl{g}b", tag="xlb")
                    nc.sync.dma_start(
                        xl_b[:], xtl[:, g * CW + HW_:(g + 1) * CW]
                        .rearrange("p (dt s) -> p dt s", s=SCH))

                    def rh_of(dt, a=xh_a, b=xh_b):
                        return (a if dt < 4 else b)[:, dt % 4, :]

                    def rl_of(dt, a=xl_a, b=xl_b):
                        return (a if dt < 4 else b)[:, dt % 4, :]
                    if g == S_CHUNKS - 1:
                        nc.sync.dma_start(identb_sb[:], identb_in[:, :])
                        nc.sync.dma_start(ut_sb[:], ut128_in[:, :])
                        nc.sync.dma_start(mcarry_sb[:], mcarry_in[:, :])
                        nc.sync.dma_start(ones64_sb[:], ones64_in[:, :])
                        nc.sync.dma_start(onescol_sb[:], onescol_in[:, :])
                        nc.sync.dma_start(iota4_sb[:], iota4_in[:, :])
                        nc.sync.dma_start(tokones3_sb[:], tokones3_in[:, :])
                    # hi and lo gate columns stacked in one stationary:
                    # ps_a rows 0:8 = gh@xh, rows 8:16 = gl@xh (one N-pass),
                    # ps_b = gh@xl; logits = sum of the three.
                    ps_a = plg_pool.tile([40, SCH], F32, name=f"ps_a{g}", tag="ps_a")
                    ps_b = plg_pool.tile([E, SCH], F32, name=f"ps_b{g}", tag="ps_b")
                    for dt in range(D_TILES):
                        nc.tensor.matmul(ps_a[:], ghl_sb[:, dt, :], rh_of(dt),
                                         start=(dt == 0), stop=(dt == D_TILES - 1),
                                         skip_group_check=True)
                        nc.tensor.matmul(ps_b[:], ghl_sb[:, dt, 0:E], rl_of(dt),
                                         start=(dt == 0), stop=(dt == D_TILES - 1),
                                         skip_group_check=True)
                    lgm = lgt_pool.tile([E, SCH], F32, name=f"lgm{g}", tag="lgm")
                    nc.scalar.copy(lgm[:], ps_a[0:E, :])
                    lgm2 = lgt_pool.tile([E, SCH], F32, name=f"lgm2{g}", tag="lgm2")
                    nc.vector.tensor_add(lgm2[:], lgm[:], ps_a[32:32 + E, :])
                    lgt_sb = lgt_pool.tile([E, SCH], F32, name=f"lgt{g}", tag="lgt")
                    nc.vector.tensor_add(lgt_sb[:], lgm2[:], ps_b[:])
                    for j in range(4):
                        st = g * 4 + j
                        ps_t2 = ptr2_pool.tile([P, E], F32, name=f"pst2{st}", tag="pst2")
                        nc.tensor.transpose(
                            ps_t2[:], lgt_sb[:, j * P:(j + 1) * P], ident_sb[:E, :E])
                        nc.vector.tensor_copy(
                            logits_all[:, st * E:(st + 1) * E], ps_t2[:])
                    # fused routing for this chunk's 4 token tiles
                    NT = 4
                    t0 = g * NT
                    lgc = logits_all[:, t0 * E:(t0 + NT) * E]
                    lg3c = lgc.rearrange("p (t e) -> p t e", e=E)
                    selc = selbig_sb[:, t0 * E:(t0 + NT) * E]
                    m1 = rts_pool.tile([P, NT], F32, name=f"m1_{g}", tag="m1")
                    nc.vector.reduce_max(m1[:], lg3c, axis=mybir.AxisListType.X)
                    mask1 = rts_pool.tile([P, NT * E], F32, name=f"mk1_{g}", tag="mk1")
                    nc.vector.tensor_tensor(
                        out=mask1[:].rearrange("p (t e) -> p t e", e=E),
                        in0=lg3c,
                        in1=m1[:, :, None].to_broadcast([P, NT, E]),
                        op=mybir.AluOpType.is_equal)
                    masked = rts_pool.tile([P, NT * E], F32, name=f"msk_{g}", tag="msk")
                    nc.vector.scalar_tensor_tensor(
                        out=masked[:], in0=mask1[:], scalar=-1e9, in1=lgc,
                        op0=mybir.AluOpType.mult, op1=mybir.AluOpType.add)
                    m2 = rts_pool.tile([P, NT], F32, name=f"m2_{g}", tag="m2")
                    nc.vector.reduce_max(
                        m2[:], masked[:].rearrange("p (t e) -> p t e", e=E),
                        axis=mybir.AxisListType.X)
                    mask2 = rts_pool.tile([P, NT * E], F32, name=f"mk2_{g}", tag="mk2")
                    nc.vector.tensor_tensor(
                        out=mask2[:].rearrange("p (t e) -> p t e", e=E),
                        in0=masked[:].rearrange("p (t e) -> p t e", e=E),
                        in1=m2[:, :, None].to_broadcast([P, NT, E]),
                        op=mybir.AluOpType.is_equal)
                    cm1 = rts_pool.tile([P, NT * E], F32, name=f"cm1_{g}", tag="cm1")
                    nc.vector.tensor_mul(cm1[:], mask1[:], selc)
                    nc.vector.reduce_sum(
                        chosen12[:, t0:t0 + NT],
                        cm1[:].rearrange("p (t e) -> p t e", e=E),
                        axis=mybir.AxisListType.X)
                    cm2 = rts_pool.tile([P, NT * E], F32, name=f"cm2_{g}", tag="cm2")
                    nc.vector.tensor_mul(cm2[:], mask2[:], selc)
                    nc.vector.reduce_sum(
                        chosen12[:, S_TILES + t0:S_TILES + t0 + NT],
                        cm2[:].rearrange("p (t e) -> p t e", e=E),
                        axis=mybir.AxisListType.X)
                    nc.vector.tensor_copy(ch_bf[:, t0:t0 + NT],
                                          chosen12[:, t0:t0 + NT])
                    nc.vector.tensor_copy(ch_bf[:, S_TILES + t0:S_TILES + t0 + NT],
                                          chosen12[:, S_TILES + t0:S_TILES + t0 + NT])

            # FFN weights + bias: queued behind the logit loads on the same
            # DMA queue so they don't steal bandwidth from the critical path.
            for dt in range(D_TILES):
                nc.sync.dma_start(wh_sb[:, dt, :], wh[dt * P:(dt + 1) * P, :])
            nc.sync.dma_start(biasbc_sb[:], biasbc_in[:, :])

            # ---- Phase R: routing ----
            with (
                tc.tile_pool(name="rt", bufs=1) as rt_pool,
                tc.tile_pool(name="prt", bufs=1, space="PSUM") as prt_pool,
            ):
                # cumsum over tokens: intra-tile (UT128) + cross-tile carries
                # (all-integer values; bf16 operands run the PE at full rate)
                ps_r = prt_pool.tile([P, 2 * S_TILES], F32, name="ps_r")
                nc.tensor.matmul(ps_r[:], ut_sb[:], ch_bf[:],
                                 start=True, stop=False, skip_group_check=True)
                ps_tot = prt_pool.tile([2 * S_TILES, 1], F32, name="ps_tot")
                nc.tensor.matmul(ps_tot[:], ch_bf[:], onescol_sb[:],
                                 start=True, stop=True)
                totcol = rt_pool.tile([2 * S_TILES, 1], F32, name="totcol")
                nc.vector.tensor_copy(totcol[:], ps_tot[:])
                rmat = rt_pool.tile([2 * S_TILES, 2 * S_TILES], BF16, name="rmat")
                nc.vector.tensor_mul(
                    rmat[:], totcol[:].to_broadcast([2 * S_TILES, 2 * S_TILES]),
                    mcarry_sb[:])
                nc.tensor.matmul(ps_r[:], ones64_sb[:], rmat[:],
                                 start=False, stop=True, skip_group_check=True)

                # slot = ch1*cum1 + ch2*cum2 - 1 + (1-ch1-ch2)*SENT
                u1 = rt_pool.tile([P, S_TILES], F32, name="u1")
                nc.vector.tensor_mul(u1[:], chosen12[:, 0:S_TILES], ps_r[:, 0:S_TILES])
                u2 = rt_pool.tile([P, S_TILES], F32, name="u2")
                nc.vector.tensor_mul(u2[:], chosen12[:, S_TILES:2 * S_TILES],
                                     ps_r[:, S_TILES:2 * S_TILES])
                # slots as 1-based cumsum ranks; unchosen tokens stay 0 and
                # never match the (c+1)-valued iota, so no sentinel is needed
                slots_f = rt_pool.tile([P, S_TILES], F32, name="slots_f")
                nc.vector.tensor_add(slots_f[:], u1[:], u2[:])

                # inverse map slot->token: fp16 one-hots (ints exact to 2048),
                # token id carried as (p, t, valid) columns, 4 tiles per is_equal.
                with (
                    tc.tile_pool(name="pinv", bufs=1, space="PSUM") as pinv_pool,
                    tc.tile_pool(name="ptr3", bufs=2, space="PSUM") as ptr3_pool,
                    tc.tile_pool(name="minv", bufs=3) as minv_pool,
                ):
                    pia = pinv_pool.tile([3, C // 2], F32, name="pia")
                    pib = pinv_pool.tile([3, C // 2], F32, name="pib")
                    for t in range(S_TILES):
                        mt = minv_pool.tile([P, C], F16, name=f"mt{t}", tag="mt")
                        nc.vector.tensor_scalar(
                            out=mt[:], in0=iota4_sb[:, 0:C],
                            scalar1=slots_f[:, t:t + 1], scalar2=None,
                            op0=mybir.AluOpType.is_equal)
                        nc.tensor.matmul(
                            pia[:], tokones3_sb[:, 3 * t:3 * t + 3],
                            mt[:, 0:C // 2],
                            start=(t == 0), stop=(t == S_TILES - 1),
                            skip_group_check=True)
                        nc.tensor.matmul(
                            pib[:], tokones3_sb[:, 3 * t:3 * t + 3],
                            mt[:, C // 2:C],
                            start=(t == 0), stop=(t == S_TILES - 1),
                            skip_group_check=True)
                    inv2 = rt_pool.tile([3, C], F32, name="inv2")
                    nc.vector.tensor_copy(inv2[:, 0:C // 2], pia[:])
                    nc.vector.tensor_copy(inv2[:, C // 2:C], pib[:])
                    inv_sb = rt_pool.tile([P, 3 * C_TILES], F32, name="inv_sb")
                    for ct in range(C_TILES):
                        pst = ptr3_pool.tile([P, 3], F32, name=f"pst{ct}", tag="pst")
                        nc.tensor.transpose(
                            pst[:], inv2[:, ct * P:(ct + 1) * P], ident_sb[:3, :3])
                        nc.vector.tensor_copy(inv_sb[:, 3 * ct:3 * ct + 3], pst[:])
                i3 = inv_sb[:].rearrange("p (c k) -> p c k", k=3)
                tokp = i3[:, :, 0:1].rearrange("p c k -> p (c k)")
                tokt = i3[:, :, 1:2].rearrange("p c k -> p (c k)")
                validv = i3[:, :, 2:3].rearrange("p c k -> p (c k)")
                # token = 128*t + p; empty slot -> S (zero row appended to xbh)
                tsum = rt_pool.tile([P, C_TILES], F32, name="tsum")
                nc.vector.scalar_tensor_tensor(
                    out=tsum[:], in0=tokt, scalar=float(P), in1=tokp,
                    op0=mybir.AluOpType.mult, op1=mybir.AluOpType.add)
                vv2 = rt_pool.tile([P, C_TILES], F32, name="vv2")
                nc.vector.scalar_tensor_tensor(
                    out=vv2[:], in0=validv, scalar=-float(S), in1=tsum[:],
                    op0=mybir.AluOpType.mult, op1=mybir.AluOpType.add)
                slf2 = rt_pool.tile([P, C_TILES], F32, name="slf2")
                nc.vector.tensor_scalar_add(slf2[:], vv2[:], float(S))
                nc.vector.tensor_copy(sl_i[:], slf2[:])

            # ---- gather; ct0 transposed on PE (fast FFN start), ct1-4 via
            # DRAM bounce + DMA transpose engine, hidden under the FFN ----
            with (
                tc.tile_pool(name="disp", bufs=5) as disp_pool,
                tc.tile_pool(name="ptp", bufs=2, space="PSUM") as ptp_pool,
                tc.tile_pool(name="po", bufs=6, space="PSUM") as po_pool,
                tc.tile_pool(name="osb", bufs=4) as osb_pool,
            ):
                disp_sbs = []
                for ct in range(C_TILES):
                    disp_sb = disp_pool.tile([P, D], BF16, name=f"disp{ct}", tag=f"d{ct}")
                    nc.gpsimd.indirect_dma_start(
                        out=disp_sb[:],
                        out_offset=None,
                        in_=xbh[:, :],
                        in_offset=bass.IndirectOffsetOnAxis(ap=sl_i[:, ct:ct + 1], axis=0),
                        bounds_check=S,
                        oob_is_err=False)
                    disp_sbs.append(disp_sb)
                for ct in range(C_TILES):
                    pt = ptp_pool.tile([P, D_TILES, P], BF16, name=f"pt{ct}", tag="pt")
                    for dt in range(D_TILES):
                        nc.tensor.transpose(
                            pt[:, dt, :], disp_sbs[ct][:, dt * P:(dt + 1) * P],
                            identb_sb[:])
                    nc.vector.tensor_copy(dh_cts[ct][:], pt[:])
                    for f in range(F_CHUNKS):
                        ps_o = po_pool.tile([P, FCH], F32, name=f"po{ct}_{f}", tag="po")
                        for dt in range(D_TILES):
                            nc.tensor.matmul(
                                ps_o[:], dh_cts[ct][:, dt, :],
                                wh_sb[:, dt, f * FCH:(f + 1) * FCH],
                                start=(dt == 0), stop=(dt == D_TILES - 1),
                                skip_group_check=True)
                        o_sb = osb_pool.tile([P, FCH], BF16, name=f"o{ct}_{f}", tag="osb")
                        nc.vector.tensor_tensor(
                            out=o_sb[:], in0=ps_o[:],
                            in1=biasbc_sb[:, f * FCH:(f + 1) * FCH],
                            op=mybir.AluOpType.add)
                        nc.sync.dma_start(
                            out[ct * P:(ct + 1) * P, f * FCH:(f + 1) * FCH], o_sb[:])

    nc.compile()
    return nc


def _consts():
    bf16 = ml_dtypes.bfloat16
    ident = np.eye(P, dtype=np.float32)
    identb = np.eye(P, dtype=bf16)
    ut128 = np.triu(np.ones((P, P), dtype=bf16))
    n = S_TILES
    slt = np.triu(np.ones((n, n), dtype=np.float32), k=1)
    mcarry = np.zeros((2 * n, 2 * n), dtype=np.float32)
    mcarry[:n, :n] = slt
    mcarry[:n, n:] = 1.0
    mcarry[n:, n:] = slt
    ones64 = np.ones((2 * n, P), dtype=bf16)
    onescol = np.ones((P, 1), dtype=bf16)
    iota4 = np.broadcast_to(
        np.tile(np.arange(1, C + 1, dtype=np.float16), 4)[None, :], (P, 4 * C)).copy()
    tokones3 = np.zeros((P, 3 * n), dtype=np.float16)
    tokones3[:, 0::3] = np.arange(P, dtype=np.float16)[:, None]
    tokones3[:, 1::3] = np.arange(n, dtype=np.float16)[None, :]
    tokones3[:, 2::3] = 1.0
    return dict(ident=ident, identb=identb, ut128=ut128, mcarry=mcarry,
                ones64=ones64, onescol=onescol, iota4=iota4, tokones3=tokones3)


def kernel(x, gate_w, weight, bias, _trace=False):
    if "nc" not in _BUILT:
        _BUILT["nc"] = _build()
    nc = _BUILT["nc"]

    bf16 = ml_dtypes.bfloat16
    xf = np.ascontiguousarray(x, dtype=np.float32)
    xbh0 = xf.astype(bf16)                                  # [S, D]
    xtl_f = xf - xbh0.astype(np.float32)
    xbh = np.zeros((S + P, D), dtype=bf16)
    xbh[:S] = xbh0

    def chunk_layout(xt):
        # [D, S] -> [p, (g, dt, s)] so each partition reads 8KB contiguously
        return np.ascontiguousarray(
            xt.reshape(D_TILES, P, S_CHUNKS, SCH).transpose(1, 2, 0, 3)
            .reshape(P, S_CHUNKS * D_TILES * SCH))

    xth = chunk_layout(xbh0.T)
    xtl = chunk_layout(xtl_f.astype(bf16).T)

    g32 = gate_w.astype(np.float32)                         # [E, D]
    gh = g32.astype(bf16)
    gl = (g32 - gh.astype(np.float32)).astype(bf16)
    ghlT = np.zeros((D, 40), dtype=bf16)   # gh in cols 0:8, gl in cols 32:40
    ghlT[:, 0:E] = gh.T
    ghlT[:, 32:32 + E] = gl.T

    bias_f = bias.reshape(E, F).astype(np.float32)
    bias_h = bias_f.astype(bf16)

    consts = _consts()

    in_maps = []
    for e in range(E):
        sel = np.zeros((P, S_TILES * E), dtype=np.float32)
        sel[:, e::E] = 1.0
        m = dict(xth=xth, xtl=xtl, xbh=xbh, ghlT=ghlT,
                 wh=np.ascontiguousarray(weight[e].astype(bf16)),
                 biasbc=np.ascontiguousarray(
                     np.broadcast_to(bias_h[e][None, :], (P, F))),
                 selbig=sel, **consts)
        in_maps.append(m)

    kw = {}
    if _trace:
        import types, sys
        from trn_agent_boot.trn_boot import _ntff_profile_via_ctypes
        hook = _ntff_profile_via_ctypes('/opt/axon/libaxon_pjrt.so')
        mod = types.ModuleType('antenv.axon_hooks')
        mod.get_axon_ntff_profile_hook = lambda: hook
        sys.modules['antenv.axon_hooks'] = mod
        kw["trace"] = True

    res = run_bass_kernel_spmd(nc, in_maps, core_ids=list(range(E)), **kw)
    _BUILT["last_res"] = res
    out = np.stack([np.asarray(res.results[e]["out"]).astype(np.float32)
                    for e in range(E)])
    if _trace:
        return out, res
    return out


# revision 32
# speedup vs baseline: 1.2168x; 1.2168x over previous
"""Trainium2 Bass kernel for nn_ExpertParallelFrontBlock (MoE top-2 routing + front FFN).

Expert-parallel: 1 expert per NeuronCore (8 cores). Only the discrete top-2
decisions matter for the output (dispatch uses the 0/1 slot mask, not softmax
probs), so no softmax is computed.

Per core:
  1. Gate logits for all 4096 tokens, exact 3-pass bf16 hi/lo (top-1/2 margins
     are ~1e-5 so the router needs fp32-grade logits). The hi and lo gate
     columns ride one stationary (PE cost is N cycles regardless of M), so it
     is 2 matmuls per d-tile, not 3. x arrives host-pre-split/pre-transposed.
     Per-chunk top-2 mask/chosen extraction is fused into the stream loop.
  2. Routing: rank-in-expert via UT128 cumsum matmul + cross-tile carry
     (bf16 operands, exact integer math); capacity drop falls out of the
     slot->token inverse map, built with fp16 one-hot matmuls whose token id
     is carried as exact (p, t, valid) columns.
  3. Dispatch: indirect-DMA row gather from bf16 x (empty slots read an
     appended zero row), transposed on the PE per c-tile.
  4. FFN [640,1024]@[1024,4096]+bias in single-pass bf16 (output tolerance is
     2e-2; 1-pass is ~4e-3) with the bias added on the vector engine during
     PSUM eviction; bf16 output is upcast on host.

Host pre-formats all operands (bf16 hi/lo splits, transposes, per-chunk
contiguous layouts, replicated bias); weights/bias/x staged per core.

self-contained: hardcodes all shapes from the problem spec.
"""
import numpy as np
import ml_dtypes

import concourse.bass as bass
import concourse.mybir as mybir
import concourse.tile as tile
from concourse import bacc
from concourse.bass_utils import run_bass_kernel_spmd

F32 = mybir.dt.float32
BF16 = mybir.dt.bfloat16
I32 = mybir.dt.int32
F16 = mybir.dt.float16

S, D, E, F = 4096, 1024, 8, 4096
C = 640  # capacity: floor(1.25*4096/8) = 640 (even)
P = 128
S_TILES = S // P      # 32
D_TILES = D // P      # 8
C_TILES = C // P      # 5
FCH = 512
F_CHUNKS = F // FCH   # 8
SCH = 512
S_CHUNKS = S // SCH   # 8
SENT = 100000.0

_BUILT = {}


def _build():
    nc = bacc.Bacc("TRN2", target_bir_lowering=False, debug=False, num_devices=E)

    xth = nc.dram_tensor("xth", [P, S_CHUNKS * D_TILES * SCH], BF16, kind="ExternalInput")
    xtl = nc.dram_tensor("xtl", [P, S_CHUNKS * D_TILES * SCH], BF16, kind="ExternalInput")
    xbh = nc.dram_tensor("xbh", [S + P, D], BF16, kind="ExternalInput")
    ghlT_in = nc.dram_tensor("ghlT", [D, 40], BF16, kind="ExternalInput")
    wh = nc.dram_tensor("wh", [D, F], BF16, kind="ExternalInput")
    biasbc_in = nc.dram_tensor("biasbc", [P, F], BF16, kind="ExternalInput")
    selbig = nc.dram_tensor("selbig", [P, S_TILES * E], F32, kind="ExternalInput")
    ident_in = nc.dram_tensor("ident", [P, P], F32, kind="ExternalInput")
    identb_in = nc.dram_tensor("identb", [P, P], BF16, kind="ExternalInput")
    ut128_in = nc.dram_tensor("ut128", [P, P], BF16, kind="ExternalInput")
    mcarry_in = nc.dram_tensor("mcarry", [2 * S_TILES, 2 * S_TILES], F32, kind="ExternalInput")
    ones64_in = nc.dram_tensor("ones64", [2 * S_TILES, P], BF16, kind="ExternalInput")
    onescol_in = nc.dram_tensor("onescol", [P, 1], BF16, kind="ExternalInput")
    iota4_in = nc.dram_tensor("iota4", [P, 4 * C], F16, kind="ExternalInput")
    tokones3_in = nc.dram_tensor("tokones3", [P, 3 * S_TILES], F16, kind="ExternalInput")
    out = nc.dram_tensor("out", [C, F], BF16, kind="ExternalOutput")

    with tile.TileContext(nc) as tc:
        with (
            tc.tile_pool(name="const", bufs=1) as cpool,
            tc.tile_pool(name="persist", bufs=1) as ppool,
        ):
            # ---- constants into SBUF ----
            ghl_sb = cpool.tile([P, D_TILES, 40], BF16, name="ghl_sb")
            nc.sync.dma_start(ghl_sb[:],
                              ghlT_in[:, :].rearrange("(dt p) e -> p dt e", p=P))
            ident_sb = cpool.tile([P, P], F32, name="ident_sb")
            nc.sync.dma_start(ident_sb[:], ident_in[:, :])
            identb_sb = cpool.tile([P, P], BF16, name="identb_sb")
            selbig_sb = cpool.tile([P, S_TILES * E], F32, name="selbig_sb")
            nc.sync.dma_start(selbig_sb[:], selbig[:, :])
            ut_sb = cpool.tile([P, P], BF16, name="ut_sb")
            mcarry_sb = cpool.tile([2 * S_TILES, 2 * S_TILES], F32, name="mcarry_sb")
            ones64_sb = cpool.tile([2 * S_TILES, P], BF16, name="ones64_sb")
            onescol_sb = cpool.tile([P, 1], BF16, name="onescol_sb")
            iota4_sb = cpool.tile([P, 4 * C], F16, name="iota4_sb")
            tokones3_sb = cpool.tile([P, 3 * S_TILES], F16, name="tokones3_sb")

            # persistent across phases
            logits_all = ppool.tile([P, S_TILES * E], F32, name="logits_all")
            chosen12 = ppool.tile([P, 2 * S_TILES], F32, name="chosen12")
            ch_bf = ppool.tile([P, 2 * S_TILES], BF16, name="ch_bf")
            wh_sb = ppool.tile([P, D_TILES, F], BF16, name="wh_sb")
            sl_i = ppool.tile([P, C_TILES], I32, name="sl_i")
            dh_cts = [ppool.tile([P, D_TILES, P], BF16, name=f"dhct{ct}")
                      for ct in range(C_TILES)]
            biasbc_sb = ppool.tile([P, F], BF16, name="biasbc_sb")

            # ---- Phase L: gate logits, 3-pass bf16 from host-transposed x ----
            with (
                tc.tile_pool(name="xt", bufs=4) as xt_pool,
                tc.tile_pool(name="xt0", bufs=1) as xt0_pool,
                tc.tile_pool(name="plg", bufs=2, space="PSUM") as plg_pool,
                tc.tile_pool(name="ptr2", bufs=2, space="PSUM") as ptr2_pool,
                tc.tile_pool(name="lgt", bufs=2) as lgt_pool,
                tc.tile_pool(name="rts", bufs=2) as rts_pool,
            ):
                CW = D_TILES * SCH
                HW_ = CW // 2
                for g in range(S_CHUNKS):
                    if g == 0:
                        # first chunk in two separate tile pairs so the PE can
                        # start while the second half is still in flight
                        xh_a = xt0_pool.tile([P, 4, SCH], BF16, name="xh0a")
                        nc.sync.dma_start(
                            xh_a[:], xth[:, 0:HW_]
                            .rearrange("p (dt s) -> p dt s", s=SCH))
                        xl_a = xt0_pool.tile([P, 4, SCH], BF16, name="xl0a")
                        nc.sync.dma_start(
                            xl_a[:], xtl[:, 0:HW_]
                            .rearrange("p (dt s) -> p dt s", s=SCH))
                        xh_b = xt0_pool.tile([P, 4, SCH], BF16, name="xh0b")
                        nc.sync.dma_start(
                            xh_b[:], xth[:, HW_:CW]
                            .rearrange("p (dt s) -> p dt s", s=SCH))
                        xl_b = xt0_pool.tile([P, 4, SCH], BF16, name="xl0b")
                        nc.sync.dma_start(
                            xl_b[:], xtl[:, HW_:CW]
                            .rearrange("p (dt s) -> p dt s", s=SCH))

                        def rh_of(dt):
                            return (xh_a if dt < 4 else xh_b)[:, dt % 4, :]

                        def rl_of(dt):
                            return (xl_a if dt < 4 else xl_b)[:, dt % 4, :]
                    else:
                        xh_t = xt_pool.tile([P, D_TILES, SCH], BF16, name=f"xh{g}", tag="xh")
                        nc.sync.dma_start(
                            xh_t[:], xth[:, g * CW:(g + 1) * CW]
                            .rearrange("p (dt s) -> p dt s", s=SCH))
                        xl_t = xt_pool.tile([P, D_TILES, SCH], BF16, name=f"xl{g}", tag="xl")
                        nc.sync.dma_start(
                            xl_t[:], xtl[:, g * CW:(g + 1) * CW]
                            .rearrange("p (dt s) -> p dt s", s=SCH))

                        def rh_of(dt, xh_t=xh_t):
                            return xh_t[:, dt, :]

                        def rl_of(dt, xl_t=xl_t):
                            return xl_t[:, dt, :]
                    if g == S_CHUNKS - 1:
                        nc.sync.dma_start(identb_sb[:], identb_in[:, :])
                        nc.sync.dma_start(ut_sb[:], ut128_in[:, :])
                        nc.sync.dma_start(mcarry_sb[:], mcarry_in[:, :])
                        nc.sync.dma_start(ones64_sb[:], ones64_in[:, :])
                        nc.sync.dma_start(onescol_sb[:], onescol_in[:, :])
                        nc.sync.dma_start(iota4_sb[:], iota4_in[:, :])
                        nc.sync.dma_start(tokones3_sb[:], tokones3_in[:, :])
                    # hi and lo gate columns stacked in one stationary:
                    # ps_a rows 0:8 = gh@xh, rows 8:16 = gl@xh (one N-pass),
                    # ps_b = gh@xl; logits = sum of the three.
                    ps_a = plg_pool.tile([40, SCH], F32, name=f"ps_a{g}", tag="ps_a")
                    ps_b = plg_pool.tile([E, SCH], F32, name=f"ps_b{g}", tag="ps_b")
                    for dt in range(D_TILES):
                        nc.tensor.matmul(ps_a[:], ghl_sb[:, dt, :], rh_of(dt),
                                         start=(dt == 0), stop=(dt == D_TILES - 1),
                                         skip_group_check=True)
                        nc.tensor.matmul(ps_b[:], ghl_sb[:, dt, 0:E], rl_of(dt),
                                         start=(dt == 0), stop=(dt == D_TILES - 1),
                                         skip_group_check=True)
                    lgm = lgt_pool.tile([E, SCH], F32, name=f"lgm{g}", tag="lgm")
                    nc.scalar.copy(lgm[:], ps_a[0:E, :])
                    lgm2 = lgt_pool.tile([E, SCH], F32, name=f"lgm2{g}", tag="lgm2")
                    nc.vector.tensor_add(lgm2[:], lgm[:], ps_a[32:32 + E, :])
                    lgt_sb = lgt_pool.tile([E, SCH], F32, name=f"lgt{g}", tag="lgt")
                    nc.vector.tensor_add(lgt_sb[:], lgm2[:], ps_b[:])
                    for j in range(4):
                        st = g * 4 + j
                        ps_t2 = ptr2_pool.tile([P, E], F32, name=f"pst2{st}", tag="pst2")
                        nc.tensor.transpose(
                            ps_t2[:], lgt_sb[:, j * P:(j + 1) * P], ident_sb[:E, :E])
                        nc.vector.tensor_copy(
                            logits_all[:, st * E:(st + 1) * E], ps_t2[:])
                    # fused routing for this chunk's 4 token tiles
                    NT = 4
                    t0 = g * NT
                    lgc = logits_all[:, t0 * E:(t0 + NT) * E]
                    lg3c = lgc.rearrange("p (t e) -> p t e", e=E)
                    selc = selbig_sb[:, t0 * E:(t0 + NT) * E]
                    m1 = rts_pool.tile([P, NT], F32, name=f"m1_{g}", tag="m1")
                    nc.vector.reduce_max(m1[:], lg3c, axis=mybir.AxisListType.X)
                    mask1 = rts_pool.tile([P, NT * E], F32, name=f"mk1_{g}", tag="mk1")
                    nc.vector.tensor_tensor(
                        out=mask1[:].rearrange("p (t e) -> p t e", e=E),
                        in0=lg3c,
                        in1=m1[:, :, None].to_broadcast([P, NT, E]),
                        op=mybir.AluOpType.is_equal)
                    masked = rts_pool.tile([P, NT * E], F32, name=f"msk_{g}", tag="msk")
                    nc.vector.scalar_tensor_tensor(
                        out=masked[:], in0=mask1[:], scalar=-1e9, in1=lgc,
                        op0=mybir.AluOpType.mult, op1=mybir.AluOpType.add)
                    m2 = rts_pool.tile([P, NT], F32, name=f"m2_{g}", tag="m2")
                    nc.vector.reduce_max(
                        m2[:], masked[:].rearrange("p (t e) -> p t e", e=E),
                        axis=mybir.AxisListType.X)
                    mask2 = rts_pool.tile([P, NT * E], F32, name=f"mk2_{g}", tag="mk2")
                    nc.vector.tensor_tensor(
                        out=mask2[:].rearrange("p (t e) -> p t e", e=E),
                        in0=masked[:].rearrange("p (t e) -> p t e", e=E),
                        in1=m2[:, :, None].to_broadcast([P, NT, E]),
                        op=mybir.AluOpType.is_equal)
                    cm1 = rts_pool.tile([P, NT * E], F32, name=f"cm1_{g}", tag="cm1")
                    nc.vector.tensor_mul(cm1[:], mask1[:], selc)
                    nc.vector.reduce_sum(
                        chosen12[:, t0:t0 + NT],
                        cm1[:].rearrange("p (t e) -> p t e", e=E),
                        axis=mybir.AxisListType.X)
                    cm2 = rts_pool.tile([P, NT * E], F32, name=f"cm2_{g}", tag="cm2")
                    nc.vector.tensor_mul(cm2[:], mask2[:], selc)
                    nc.vector.reduce_sum(
                        chosen12[:, S_TILES + t0:S_TILES + t0 + NT],
                        cm2[:].rearrange("p (t e) -> p t e", e=E),
                        axis=mybir.AxisListType.X)
                    nc.vector.tensor_copy(ch_bf[:, t0:t0 + NT],
                                          chosen12[:, t0:t0 + NT])
                    nc.vector.tensor_copy(ch_bf[:, S_TILES + t0:S_TILES + t0 + NT],
                                          chosen12[:, S_TILES + t0:S_TILES + t0 + NT])

            # FFN weights + bias: queued behind the logit loads on the same
            # DMA queue so they don't steal bandwidth from the critical path.
            for dt in range(D_TILES):
                nc.sync.dma_start(wh_sb[:, dt, :], wh[dt * P:(dt + 1) * P, :])
            nc.sync.dma_start(biasbc_sb[:], biasbc_in[:, :])

            # ---- Phase R: routing ----
            with (
                tc.tile_pool(name="rt", bufs=1) as rt_pool,
                tc.tile_pool(name="prt", bufs=1, space="PSUM") as prt_pool,
            ):
                # cumsum over tokens: intra-tile (UT128) + cross-tile carries
                # (all-integer values; bf16 operands run the PE at full rate)
                ps_r = prt_pool.tile([P, 2 * S_TILES], F32, name="ps_r")
                nc.tensor.matmul(ps_r[:], ut_sb[:], ch_bf[:],
                                 start=True, stop=False, skip_group_check=True)
                ps_tot = prt_pool.tile([2 * S_TILES, 1], F32, name="ps_tot")
                nc.tensor.matmul(ps_tot[:], ch_bf[:], onescol_sb[:],
                                 start=True, stop=True)
                totcol = rt_pool.tile([2 * S_TILES, 1], F32, name="totcol")
                nc.vector.tensor_copy(totcol[:], ps_tot[:])
                rmat = rt_pool.tile([2 * S_TILES, 2 * S_TILES], BF16, name="rmat")
                nc.vector.tensor_mul(
                    rmat[:], totcol[:].to_broadcast([2 * S_TILES, 2 * S_TILES]),
                    mcarry_sb[:])
                nc.tensor.matmul(ps_r[:], ones64_sb[:], rmat[:],
                                 start=False, stop=True, skip_group_check=True)

                # slot = ch1*cum1 + ch2*cum2 - 1 + (1-ch1-ch2)*SENT
                u1 = rt_pool.tile([P, S_TILES], F32, name="u1")
                nc.vector.tensor_mul(u1[:], chosen12[:, 0:S_TILES], ps_r[:, 0:S_TILES])
                u2 = rt_pool.tile([P, S_TILES], F32, name="u2")
                nc.vector.tensor_mul(u2[:], chosen12[:, S_TILES:2 * S_TILES],
                                     ps_r[:, S_TILES:2 * S_TILES])
                # slots as 1-based cumsum ranks; unchosen tokens stay 0 and
                # never match the (c+1)-valued iota, so no sentinel is needed
                slots_f = rt_pool.tile([P, S_TILES], F32, name="slots_f")
                nc.vector.tensor_add(slots_f[:], u1[:], u2[:])

                # inverse map slot->token: fp16 one-hots (ints exact to 2048),
                # token id carried as (p, t, valid) columns, 4 tiles per is_equal.
                with (
                    tc.tile_pool(name="pinv", bufs=1, space="PSUM") as pinv_pool,
                    tc.tile_pool(name="ptr3", bufs=2, space="PSUM") as ptr3_pool,
                    tc.tile_pool(name="minv", bufs=3) as minv_pool,
                ):
                    pia = pinv_pool.tile([3, C // 2], F32, name="pia")
                    pib = pinv_pool.tile([3, C // 2], F32, name="pib")
                    for t in range(S_TILES):
                        mt = minv_pool.tile([P, C], F16, name=f"mt{t}", tag="mt")
                        nc.vector.tensor_scalar(
                            out=mt[:], in0=iota4_sb[:, 0:C],
                            scalar1=slots_f[:, t:t + 1], scalar2=None,
                            op0=mybir.AluOpType.is_equal)
                        nc.tensor.matmul(
                            pia[:], tokones3_sb[:, 3 * t:3 * t + 3],
                            mt[:, 0:C // 2],
                            start=(t == 0), stop=(t == S_TILES - 1),
                            skip_group_check=True)
                        nc.tensor.matmul(
                            pib[:], tokones3_sb[:, 3 * t:3 * t + 3],
                            mt[:, C // 2:C],
                            start=(t == 0), stop=(t == S_TILES - 1),
                            skip_group_check=True)
                    inv2 = rt_pool.tile([3, C], F32, name="inv2")
                    nc.vector.tensor_copy(inv2[:, 0:C // 2], pia[:])
                    nc.vector.tensor_copy(inv2[:, C // 2:C], pib[:])
                    inv_sb = rt_pool.tile([P, 3 * C_TILES], F32, name="inv_sb")
                    for ct in range(C_TILES):
                        pst = ptr3_pool.tile([P, 3], F32, name=f"pst{ct}", tag="pst")
                        nc.tensor.transpose(
                            pst[:], inv2[:, ct * P:(ct + 1) * P], ident_sb[:3, :3])
                        nc.vector.tensor_copy(inv_sb[:, 3 * ct:3 * ct + 3], pst[:])
                i3 = inv_sb[:].rearrange("p (c k) -> p c k", k=3)
                tokp = i3[:, :, 0:1].rearrange("p c k -> p (c k)")
                tokt = i3[:, :, 1:2].rearrange("p c k -> p (c k)")
                validv = i3[:, :, 2:3].rearrange("p c k -> p (c k)")
                # token = 128*t + p; empty slot -> S (zero row appended to xbh)
                tsum = rt_pool.tile([P, C_TILES], F32, name="tsum")
                nc.vector.scalar_tensor_tensor(
                    out=tsum[:], in0=tokt, scalar=float(P), in1=tokp,
                    op0=mybir.AluOpType.mult, op1=mybir.AluOpType.add)
                vv2 = rt_pool.tile([P, C_TILES], F32, name="vv2")
                nc.vector.scalar_tensor_tensor(
                    out=vv2[:], in0=validv, scalar=-float(S), in1=tsum[:],
                    op0=mybir.AluOpType.mult, op1=mybir.AluOpType.add)
                slf2 = rt_pool.tile([P, C_TILES], F32, name="slf2")
                nc.vector.tensor_scalar_add(slf2[:], vv2[:], float(S))
                nc.vector.tensor_copy(sl_i[:], slf2[:])

            # ---- gather; ct0 transposed on PE (fast FFN start), ct1-4 via
            # DRAM bounce + DMA transpose engine, hidden under the FFN ----
            with (
                tc.tile_pool(name="disp", bufs=5) as disp_pool,
                tc.tile_pool(name="ptp", bufs=2, space="PSUM") as ptp_pool,
                tc.tile_pool(name="po", bufs=6, space="PSUM") as po_pool,
                tc.tile_pool(name="osb", bufs=4) as osb_pool,
            ):
                disp_sbs = []
                for ct in range(C_TILES):
                    disp_sb = disp_pool.tile([P, D], BF16, name=f"disp{ct}", tag=f"d{ct}")
                    nc.gpsimd.indirect_dma_start(
                        out=disp_sb[:],
                        out_offset=None,
                        in_=xbh[:, :],
                        in_offset=bass.IndirectOffsetOnAxis(ap=sl_i[:, ct:ct + 1], axis=0),
                        bounds_check=S,
                        oob_is_err=False)
                    disp_sbs.append(disp_sb)
                for ct in range(C_TILES):
                    pt = ptp_pool.tile([P, D_TILES, P], BF16, name=f"pt{ct}", tag="pt")
                    for dt in range(D_TILES):
                        nc.tensor.transpose(
                            pt[:, dt, :], disp_sbs[ct][:, dt * P:(dt + 1) * P],
                            identb_sb[:])
                    nc.vector.tensor_copy(dh_cts[ct][:], pt[:])
                    for f in range(F_CHUNKS):
                        ps_o = po_pool.tile([P, FCH], F32, name=f"po{ct}_{f}", tag="po")
                        for dt in range(D_TILES):
                            nc.tensor.matmul(
                                ps_o[:], dh_cts[ct][:, dt, :],
                                wh_sb[:, dt, f * FCH:(f + 1) * FCH],
                                start=(dt == 0), stop=(dt == D_TILES - 1),
                                skip_group_check=True)
                        o_sb = osb_pool.tile([P, FCH], BF16, name=f"o{ct}_{f}", tag="osb")
                        nc.vector.tensor_tensor(
                            out=o_sb[:], in0=ps_o[:],
                            in1=biasbc_sb[:, f * FCH:(f + 1) * FCH],
                            op=mybir.AluOpType.add)
                        nc.sync.dma_start(
                            out[ct * P:(ct + 1) * P, f * FCH:(f + 1) * FCH], o_sb[:])

    nc.compile()
    return nc


def _consts():
    bf16 = ml_dtypes.bfloat16
    ident = np.eye(P, dtype=np.float32)
    identb = np.eye(P, dtype=bf16)
    ut128 = np.triu(np.ones((P, P), dtype=bf16))
    n = S_TILES
    slt = np.triu(np.ones((n, n), dtype=np.float32), k=1)
    mcarry = np.zeros((2 * n, 2 * n), dtype=np.float32)
    mcarry[:n, :n] = slt
    mcarry[:n, n:] = 1.0
    mcarry[n:, n:] = slt
    ones64 = np.ones((2 * n, P), dtype=bf16)
    onescol = np.ones((P, 1), dtype=bf16)
    iota4 = np.broadcast_to(
        np.tile(np.arange(1, C + 1, dtype=np.float16), 4)[None, :], (P, 4 * C)).copy()
    tokones3 = np.zeros((P, 3 * n), dtype=np.float16)
    tokones3[:, 0::3] = np.arange(P, dtype=np.float16)[:, None]
    tokones3[:, 1::3] = np.arange(n, dtype=np.float16)[None, :]
    tokones3[:, 2::3] = 1.0
    return dict(ident=ident, identb=identb, ut128=ut128, mcarry=mcarry,
                ones64=ones64, onescol=onescol, iota4=iota4, tokones3=tokones3)


def kernel(x, gate_w, weight, bias, _trace=False):
    if "nc" not in _BUILT:
        _BUILT["nc"] = _build()
    nc = _BUILT["nc"]

    bf16 = ml_dtypes.bfloat16
    xf = np.ascontiguousarray(x, dtype=np.float32)
    xbh0 = xf.astype(bf16)                                  # [S, D]
    xtl_f = xf - xbh0.astype(np.float32)
    xbh = np.zeros((S + P, D), dtype=bf16)
    xbh[:S] = xbh0

    def chunk_layout(xt):
        # [D, S] -> [p, (g, dt, s)] so each partition reads 8KB contiguously
        return np.ascontiguousarray(
            xt.reshape(D_TILES, P, S_CHUNKS, SCH).transpose(1, 2, 0, 3)
            .reshape(P, S_CHUNKS * D_TILES * SCH))

    xth = chunk_layout(xbh0.T)
    xtl = chunk_layout(xtl_f.astype(bf16).T)

    g32 = gate_w.astype(np.float32)                         # [E, D]
    gh = g32.astype(bf16)
    gl = (g32 - gh.astype(np.float32)).astype(bf16)
    ghlT = np.zeros((D, 40), dtype=bf16)   # gh in cols 0:8, gl in cols 32:40
    ghlT[:, 0:E] = gh.T
    ghlT[:, 32:32 + E] = gl.T

    bias_f = bias.reshape(E, F).astype(np.float32)
    bias_h = bias_f.astype(bf16)

    consts = _consts()

    in_maps = []
    for e in range(E):
        sel = np.zeros((P, S_TILES * E), dtype=np.float32)
        sel[:, e::E] = 1.0
        m = dict(xth=xth, xtl=xtl, xbh=xbh, ghlT=ghlT,
                 wh=np.ascontiguousarray(weight[e].astype(bf16)),
                 biasbc=np.ascontiguousarray(
                     np.broadcast_to(bias_h[e][None, :], (P, F))),
                 selbig=sel, **consts)
        in_maps.append(m)

    kw = {}
    if _trace:
        import types, sys
        from trn_agent_boot.trn_boot import _ntff_profile_via_ctypes
        hook = _ntff_profile_via_ctypes('/opt/axon/libaxon_pjrt.so')
        mod = types.ModuleType('antenv.axon_hooks')
        mod.get_axon_ntff_profile_hook = lambda: hook
        sys.modules['antenv.axon_hooks'] = mod
        kw["trace"] = True

    res = run_bass_kernel_spmd(nc, in_maps, core_ids=list(range(E)), **kw)
    _BUILT["last_res"] = res
    out = np.stack([np.asarray(res.results[e]["out"]).astype(np.float32)
                    for e in range(E)])
    if _trace:
        return out, res
    return out
